# revision 38
# baseline (speedup 1.0000x reference)
"""CPQuadRankLayer Trainium2 kernel, bf16 datapath, host-prepacked layouts.

Math (per node n, batch b):
  P[b,c,r]  = sum_i x[b,n,c,i] * factors[c,n,r,i]
  p         = P / sqrt(mean_r P^2 + eps)
  merged    = p0*p1*p2*p3 * gain[n]
  out[b,o]  = sum_r merged[b,r] * factor_out[n,r,o] + mean_c x[b,n,c,o]

Distribution: nodes sharded 1024 -> 8 cores x 128 nodes (node-
independent: no collectives). All tensors are repacked host-side to
bf16 with >=2KiB contiguous runs so DMA traffic is halved vs fp32 and
every matmul runs at the 1-cycle/row bf16 rate. The residual mean_c x
is folded into the output PSUM accumulation as four identity-matmul
accumulations (0.25*I stationary), which removes the entire vector-add
chain. The quartic pair-products run on the otherwise idle GpSimd
engine. Output is stored bf16 and upcast on the host.
"""

import numpy as np
import ml_dtypes

BF16 = ml_dtypes.bfloat16

B = 64
N = 1024
C = 4
D = 128
R = 64
NCORES = 8
NS = N // NCORES  # nodes per core (128)
G = 16  # nodes per group
NH = NS // 2  # node pairs per core
GH = G // 2  # node pairs per group (8)
NG = NS // G  # groups per core (8)
OCT = NS // 8  # octets per core (16)
EPS = 1e-6

_CACHE = {}


def _build_nc(repeat=1):
    import concourse.bacc as bacc
    import concourse.tile as tile
    import concourse.mybir as mybir
    from concourse.masks import make_identity

    f32 = mybir.dt.float32
    bf16 = mybir.dt.bfloat16
    f16 = mybir.dt.float16
    Act = mybir.ActivationFunctionType

    nc = bacc.Bacc()
    # x pre-packed: [group, c, i, (node16, b)] bf16 -> 2KiB runs
    xp = nc.declare_dram_parameter("xp", [NG, C, D, 1024], bf16, isOutput=False)
    # factors pre-packed: [c, group, i, (node16, r)] bf16 -> 2KiB runs
    f = nc.declare_dram_parameter("factors_t", [C, NG, D, 1024], bf16, isOutput=False)
    # factor_out pre-packed: [octet, r, (node8, o)] bf16 -> 2KiB runs
    fo = nc.declare_dram_parameter("factor_out_t", [OCT, R, 8 * D], bf16, isOutput=False)
    gain = nc.declare_dram_parameter("gain", [NS, 1], f32, isOutput=False)
    # packed output: [group, o, (gh, g2, b)] bf16; host unpacks + upcasts
    out = nc.declare_dram_parameter("out_t", [NG, 128, GH * D], bf16, isOutput=True)

    xp_r = xp.rearrange("g c i w -> i g c w")
    f_r = f.rearrange("c g i w -> i g c w")
    fo_r = fo.rearrange("u r w -> r u w")

    with tile.TileContext(nc) as tc:
        with (
            tc.tile_pool(name="consts", bufs=1) as consts,
            tc.tile_pool(name="xpool", bufs=4) as xpool,
            tc.tile_pool(name="fpool", bufs=4) as fpool,
            tc.tile_pool(name="fopool", bufs=4) as fopool,
            tc.tile_pool(name="ppcpool", bufs=2) as ppcpool,
            tc.tile_pool(name="sqpool", bufs=2) as sqpool,
            tc.tile_pool(name="mpool", bufs=2) as mpool,
            tc.tile_pool(name="mtpool", bufs=4) as mtpool,
            tc.tile_pool(name="otpool", bufs=2) as otpool,
            tc.tile_pool(name="small", bufs=10) as small,
            tc.tile_pool(name="pps", bufs=3, space="PSUM") as pps,
            tc.tile_pool(name="mtps", bufs=2, space="PSUM") as mtps,
            tc.tile_pool(name="ops", bufs=3, space="PSUM") as ops,
        ):
            identity = consts.tile([128, 128], bf16)
            make_identity(nc, identity)
            # 0.25*I stationary for the residual mean_c accumulation
            idq = consts.tile([128, 128], bf16)
            nc.scalar.mul(idq, identity, 0.25)
            eps_t = consts.tile([128, 1], f32)
            nc.vector.memset(eps_t, EPS)

            # gpair[p, h] = gain[2h + (p >= 64)] via two K=1 outer products
            ones1 = consts.tile([1, 128], f32)
            nc.vector.memset(ones1, 1.0)
            g1 = consts.tile([1, NS], f32)
            nc.sync.dma_start(out=g1, in_=gain.rearrange("n o -> o n"))
            g1v = g1.rearrange("o (h g2) -> o h g2", g2=2)
            gpp = pps.tile([128, NH], f32, tag="pp")
            nc.tensor.matmul(gpp[0:64, :], lhsT=ones1[:, 0:64], rhs=g1v[:, :, 0])
            nc.tensor.matmul(gpp[64:128, :], lhsT=ones1[:, 0:64], rhs=g1v[:, :, 1])
            gpair = consts.tile([128, NH], f32)
            nc.any.tensor_copy(gpair, gpp)

            def load(gi):
                st = {}
                xt = xpool.tile([128, C, 1024], bf16, tag="xt")
                nc.sync.dma_start(out=xt, in_=xp_r[:, gi])
                ft = fpool.tile([128, C, 1024], bf16, tag="ft")
                nc.scalar.dma_start(out=ft, in_=f_r[:, gi])
                fot = fopool.tile([R, 2, 8 * D], bf16, tag="fot")
                nc.scalar.dma_start(out=fot, in_=fo_r[:, 2 * gi : 2 * gi + 2])
                st["x"], st["f"], st["fo"] = xt, ft, fot
                return st

            def phase1_stats(gi, st):
                h0 = gi * GH
                # ssq[p, ch, c, dg]
                ssq = small.tile([128, 4, C, 2], f16, tag="ssq")
                mgall = mpool.tile([128, GH, R], bf16, tag="mgall")
                sq_prev = None

                def emit_reduce(ch, sq):
                    with nc.allow_low_precision("ssq fits fp16 comfortably"):
                        nc.vector.reduce_sum(
                            out=ssq[:, ch],
                            in_=sq.rearrange("p c dg r -> p (c dg) r"),
                            axis=mybir.AxisListType.X,
                        )

                for ch in range(4):
                    # pp[p, c, dg, r] so downstream slices are contiguous
                    pp = pps.tile([128, C, 2, R], f32, tag="pp")
                    for dg in range(2):
                        gh = 2 * ch + dg
                        for c in range(C):
                            for g2 in range(2):
                                j = 2 * gh + g2
                                nc.tensor.matmul(
                                    pp[64 * g2 : 64 * g2 + 64, c, dg, :],
                                    lhsT=st["x"][:, c, 64 * j : 64 * j + 64],
                                    rhs=st["f"][:, c, 64 * j : 64 * j + 64],
                                )
                    # single psum->sbuf eviction (bf16); all stats read SBUF
                    # (gpsimd cannot touch PSUM, DVE only one PSUM operand)
                    ppc = ppcpool.tile([128, C, 2, R], bf16, tag="ppc")
                    nc.scalar.copy(out=ppc, in_=pp)
                    # squares on gpsimd; fp16 keeps the reduce chain 2-byte
                    sq = sqpool.tile([128, C, 2, R], f16, tag="sq")
                    nc.gpsimd.tensor_mul(sq, ppc, ppc)
                    # both pair products in one 2-byte-mode op; the PREVIOUS
                    # chunk's reduce (input long ready) is emitted between the
                    # product and its consumer so the m0123 write settles
                    # without stalling the in-order vector queue
                    ppv = ppc.rearrange("p (cp ct) dg r -> p cp ct dg r", ct=2)
                    m0123 = mpool.tile([128, 2, 2, R], bf16, tag="m0123")
                    nc.vector.tensor_mul(m0123, ppv[:, :, 0], ppv[:, :, 1])
                    if sq_prev is not None:
                        emit_reduce(ch - 1, sq_prev)
                    nc.vector.tensor_mul(
                        mgall[:, 2 * ch : 2 * ch + 2, :], m0123[:, 0], m0123[:, 1]
                    )
                    sq_prev = sq
                emit_reduce(3, sq_prev)
                rms = small.tile([128, 4 * C * 2], f32, tag="rms")
                nc.scalar.activation(
                    out=rms,
                    in_=ssq.rearrange("p ch c dg -> p (ch c dg)"),
                    func=Act.Sqrt,
                    bias=eps_t,
                    scale=1.0 / R,
                )
                rstd = small.tile([128, 4, C, 2], f32, tag="rstd")
                nc.vector.reciprocal(
                    out=rstd, in_=rms.rearrange("p (ch c dg) -> p ch c dg", c=C, dg=2)
                )
                # scl2[p, gh] = gain * prod_c rstd ; gh = 2*ch + dg
                scl2a = small.tile([128, GH], f32, tag="scl2a")
                nc.vector.tensor_reduce(
                    out=scl2a,
                    in_=rstd.rearrange("p ch c dg -> p ch dg c"),
                    axis=mybir.AxisListType.X,
                    op=mybir.AluOpType.mult,
                )
                scl2 = small.tile([128, GH], f32, tag="scl2")
                nc.vector.tensor_mul(scl2, scl2a, gpair[:, h0 : h0 + GH])
                mg = mpool.tile([128, GH, R], bf16, tag="mg")
                scl2b = scl2.unsqueeze(2).broadcast_to([128, GH, R])
                nc.vector.tensor_mul(mg, mgall, scl2b)
                st["mg"] = mg

            def phase2(gi, st):
                o_t = otpool.tile([128, GH * D], bf16, tag="o_t")
                for half in range(2):
                    # one psum bank per half-group of nodes
                    op_ps = ops.tile([128, 512], f32, tag="op")
                    # residual: out += 0.25 * sum_c x, via 0.25*I stationary.
                    # c==0 starts (overwrites) the bank; FO matmuls then
                    # accumulate on top.
                    for c in range(C):
                        nc.tensor.matmul(
                            op_ps,
                            lhsT=idq,
                            rhs=st["x"][:, c, 512 * half : 512 * half + 512],
                            start=(c == 0),
                            stop=False,
                        )
                    mtp = mtps.tile([64, 512], bf16, tag="mtp")
                    for k in range(4):
                        gh = 4 * half + k
                        nc.tensor.matmul(
                            mtp[:, 128 * k : 128 * k + 128],
                            lhsT=st["mg"][:, gh, :],
                            rhs=identity,
                            is_transpose=True,
                        )
                    mt = mtpool.tile([64, 512], bf16, tag="mt")
                    nc.scalar.copy(out=mt, in_=mtp)
                    for k in range(4):
                        gh = 4 * half + k
                        for g2 in range(2):
                            j = 2 * gh + g2
                            u8, j8 = j // 8, j % 8
                            col = 64 * (2 * k + g2)
                            nc.tensor.matmul(
                                op_ps[:, col : col + 64],
                                lhsT=st["fo"][:, u8, 128 * j8 : 128 * j8 + 128],
                                rhs=mt[:, col : col + 64],
                                start=False,
                                stop=(k == 3 and g2 == 1),
                            )
                    nc.scalar.copy(
                        out=o_t[:, 512 * half : 512 * half + 512], in_=op_ps
                    )
                # all stores go out on the gpsimd SWDGE ring so the in-order
                # HWDGE load rings are never blocked behind a store that is
                # waiting on its eviction
                nc.gpsimd.dma_start(out=out[gi], in_=o_t)

            # modulo software pipeline: loads run 2 groups ahead and
            # phase1+stats one group ahead of phase2, wrapping across the
            # repeat-loop boundary so the DMA queues never drain. The wrap
            # is legal because every loop iteration reads the same DRAM,
            # and stationary because every cross-step tag's buffer count
            # divides its per-iteration allocation count, so the prologue
            # bindings coincide with the wrapped body bindings.
            states = {}

            def emit_body():
                for gi in range(NG):
                    states[(gi + 2) % NG] = load((gi + 2) % NG)
                    phase1_stats((gi + 1) % NG, states[(gi + 1) % NG])
                    phase2(gi, states[gi])

            states[0] = load(0)
            states[1] = load(1)
            phase1_stats(0, states[0])
            if repeat > 1:
                with tc.For_i(0, repeat, 1):
                    emit_body()
            else:
                emit_body()

    nc.compile()
    return nc


def _get_nc(repeat=1):
    key = ("nc", repeat)
    if key not in _CACHE:
        _CACHE[key] = _build_nc(repeat)
    return _CACHE[key]


def _pack_x(x):
    # [B, N, C, D] -> [N//16, C, D, 1024] bf16 ; n = g*16 + j, col = j*64 + b
    a = np.asarray(x).reshape(B, N // 16, 16, C, D)
    a = np.transpose(a, (1, 3, 4, 2, 0))  # [g, c, i, j, b]
    return np.ascontiguousarray(a.reshape(N // 16, C, D, 1024)).astype(BF16)


def _pack_factors(factors):
    # [4, N, R, D] -> [C, N//16, D, 1024] bf16
    f = np.asarray(factors).reshape(C, N // 16, 16, R, D)
    f = np.transpose(f, (0, 1, 4, 2, 3))  # [c, g, i, j, r]
    return np.ascontiguousarray(f.reshape(C, N // 16, D, 1024)).astype(BF16)


def _pack_factor_out(factor_out):
    # [N, R, D] -> [N//8, R, 8*D] bf16
    q = np.asarray(factor_out).reshape(N // 8, 8, R, D)
    q = np.transpose(q, (0, 2, 1, 3))  # [oct, r, node8, o]
    return np.ascontiguousarray(q.reshape(N // 8, R, 8 * D)).astype(BF16)


def _unpack_out(res_t):
    # [NG, 128(o), GH*D] bf16 with col = gh*128 + g2*64 + b -> [B, NS, D] f32
    a = np.asarray(res_t).astype(np.float32).reshape(NG, 128, GH, 2, 64)
    a = np.transpose(a, (4, 0, 2, 3, 1))  # [b, gi, gh, g2, o]
    return np.ascontiguousarray(a.reshape(64, NS, D))


def kernel(x, factors, factor_out, gain):
    from concourse.bass_utils import run_bass_kernel_spmd

    nc = _get_nc()
    x_packed = _pack_x(x)
    f_packed = _pack_factors(factors)
    fo_packed = _pack_factor_out(factor_out)
    gain = np.ascontiguousarray(np.asarray(gain, dtype=np.float32))
    in_maps = []
    for k in range(NCORES):
        lo, hi = k * NS, (k + 1) * NS
        in_maps.append(
            {
                "xp": np.ascontiguousarray(x_packed[k * NG : (k + 1) * NG]),
                "factors_t": np.ascontiguousarray(f_packed[:, k * NG : (k + 1) * NG]),
                "factor_out_t": np.ascontiguousarray(
                    fo_packed[k * OCT : (k + 1) * OCT]
                ),
                "gain": gain[lo:hi],
            }
        )
    res = run_bass_kernel_spmd(nc, in_maps, core_ids=list(range(NCORES)))
    return np.concatenate(
        [_unpack_out(res.results[k]["out_t"]) for k in range(NCORES)], axis=1
    )


# revision 42
# speedup vs baseline: 1.0769x; 1.0769x over previous
"""CPQuadRankLayer Trainium2 kernel, bf16 datapath, host-prepacked layouts.

Math (per node n, batch b):
  P[b,c,r]  = sum_i x[b,n,c,i] * factors[c,n,r,i]
  p         = P / sqrt(mean_r P^2 + eps)
  merged    = p0*p1*p2*p3 * gain[n]
  out[b,o]  = sum_r merged[b,r] * factor_out[n,r,o] + mean_c x[b,n,c,o]

Distribution: nodes sharded 1024 -> 8 cores x 128 nodes (node-
independent: no collectives). All tensors are repacked host-side to
bf16 with >=2KiB contiguous runs so DMA traffic is halved vs fp32 and
every matmul runs at the 1-cycle/row bf16 rate. The residual mean_c x
is folded into the output PSUM accumulation as four identity-matmul
accumulations (0.25*I stationary), which removes the entire vector-add
chain. The quartic pair-products run on the otherwise idle GpSimd
engine. Output is stored bf16 and upcast on the host.
"""

import numpy as np
import ml_dtypes

BF16 = ml_dtypes.bfloat16

B = 64
N = 1024
C = 4
D = 128
R = 64
NCORES = 8
NS = N // NCORES  # nodes per core (128)
G = 16  # nodes per group
NH = NS // 2  # node pairs per core
GH = G // 2  # node pairs per group (8)
NG = NS // G  # groups per core (8)
OCT = NS // 8  # octets per core (16)
EPS = 1e-6

_CACHE = {}


def _build_nc(repeat=1):
    import concourse.bacc as bacc
    import concourse.tile as tile
    import concourse.mybir as mybir
    from concourse.masks import make_identity

    f32 = mybir.dt.float32
    bf16 = mybir.dt.bfloat16
    f16 = mybir.dt.float16
    Act = mybir.ActivationFunctionType

    nc = bacc.Bacc()
    # x pre-packed: [group, c, i, (node16, b)] bf16 -> 2KiB runs
    xp = nc.declare_dram_parameter("xp", [NG, C, D, 1024], bf16, isOutput=False)
    # factors pre-packed: [c, group, i, (node16, r)] bf16 -> 2KiB runs
    f = nc.declare_dram_parameter("factors_t", [C, NG, D, 1024], bf16, isOutput=False)
    # factor_out pre-packed: [octet, r, (node8, o)] bf16 -> 2KiB runs
    fo = nc.declare_dram_parameter("factor_out_t", [OCT, R, 8 * D], bf16, isOutput=False)
    gain = nc.declare_dram_parameter("gain", [NS, 1], f32, isOutput=False)
    # packed output: [group, o, (gh, g2, b)] bf16; host unpacks + upcasts
    out = nc.declare_dram_parameter("out_t", [NG, 128, GH * D], bf16, isOutput=True)

    xp_r = xp.rearrange("g c i w -> i g c w")
    f_r = f.rearrange("c g i w -> i g c w")
    fo_r = fo.rearrange("u r w -> r u w")

    with tile.TileContext(nc) as tc:
        with (
            tc.tile_pool(name="consts", bufs=1) as consts,
            tc.tile_pool(name="xpool", bufs=4) as xpool,
            tc.tile_pool(name="fpool", bufs=4) as fpool,
            tc.tile_pool(name="fopool", bufs=4) as fopool,
            tc.tile_pool(name="ppcpool", bufs=2) as ppcpool,
            tc.tile_pool(name="sqpool", bufs=2) as sqpool,
            tc.tile_pool(name="mpool", bufs=2) as mpool,
            tc.tile_pool(name="mtpool", bufs=4) as mtpool,
            tc.tile_pool(name="otpool", bufs=2) as otpool,
            tc.tile_pool(name="small", bufs=10) as small,
            tc.tile_pool(name="pps", bufs=3, space="PSUM") as pps,
            tc.tile_pool(name="mtps", bufs=2, space="PSUM") as mtps,
            tc.tile_pool(name="ops", bufs=3, space="PSUM") as ops,
        ):
            identity = consts.tile([128, 128], bf16)
            make_identity(nc, identity)
            # 0.25*I stationary for the residual mean_c accumulation
            idq = consts.tile([128, 128], bf16)
            nc.scalar.mul(idq, identity, 0.25)
            eps_t = consts.tile([128, 1], f32)
            nc.vector.memset(eps_t, EPS)

            # gpair[p, h] = gain[2h + (p >= 64)] via two K=1 outer products
            ones1 = consts.tile([1, 128], f32)
            nc.vector.memset(ones1, 1.0)
            g1 = consts.tile([1, NS], f32)
            nc.sync.dma_start(out=g1, in_=gain.rearrange("n o -> o n"))
            g1v = g1.rearrange("o (h g2) -> o h g2", g2=2)
            gpp = pps.tile([128, NH], f32, tag="pp")
            nc.tensor.matmul(gpp[0:64, :], lhsT=ones1[:, 0:64], rhs=g1v[:, :, 0])
            nc.tensor.matmul(gpp[64:128, :], lhsT=ones1[:, 0:64], rhs=g1v[:, :, 1])
            gpair = consts.tile([128, NH], f32)
            nc.any.tensor_copy(gpair, gpp)

            def load(gi):
                st = {}
                xt = xpool.tile([128, C, 1024], bf16, tag="xt")
                nc.sync.dma_start(out=xt, in_=xp_r[:, gi])
                ft = fpool.tile([128, C, 1024], bf16, tag="ft")
                nc.scalar.dma_start(out=ft, in_=f_r[:, gi])
                fot = fopool.tile([R, 2, 8 * D], bf16, tag="fot")
                nc.scalar.dma_start(out=fot, in_=fo_r[:, 2 * gi : 2 * gi + 2])
                st["x"], st["f"], st["fo"] = xt, ft, fot
                return st

            def phase1_stats(gi, st):
                h0 = gi * GH
                # ssq[p, ch, c, dg]
                ssq = small.tile([128, 4, C, 2], f16, tag="ssq")
                mgall = mpool.tile([128, GH, R], bf16, tag="mgall")
                sq_prev = None
                m_prev = None

                def emit_reduce(ch, sq):
                    with nc.allow_low_precision("ssq fits fp16 comfortably"):
                        nc.vector.reduce_sum(
                            out=ssq[:, ch],
                            in_=sq.rearrange("p c dg r -> p (c dg) r"),
                            axis=mybir.AxisListType.X,
                        )

                def emit_quartic(ch, m0123):
                    nc.vector.tensor_mul(
                        mgall[:, 2 * ch : 2 * ch + 2, :], m0123[:, 0], m0123[:, 1]
                    )

                for ch in range(4):
                    # pp[p, c, dg, r] so downstream slices are contiguous
                    pp = pps.tile([128, C, 2, R], f32, tag="pp")
                    for dg in range(2):
                        gh = 2 * ch + dg
                        for c in range(C):
                            for g2 in range(2):
                                j = 2 * gh + g2
                                nc.tensor.matmul(
                                    pp[64 * g2 : 64 * g2 + 64, c, dg, :],
                                    lhsT=st["x"][:, c, 64 * j : 64 * j + 64],
                                    rhs=st["f"][:, c, 64 * j : 64 * j + 64],
                                )
                    # single psum->sbuf eviction (bf16); all stats read SBUF
                    # (gpsimd cannot touch PSUM, DVE only one PSUM operand)
                    ppc = ppcpool.tile([128, C, 2, R], bf16, tag="ppc")
                    nc.scalar.copy(out=ppc, in_=pp)
                    # squares on gpsimd; fp16 keeps the reduce chain 2-byte
                    sq = sqpool.tile([128, C, 2, R], f16, tag="sq")
                    nc.gpsimd.tensor_mul(sq, ppc, ppc)
                    # both pair products in one 2-byte-mode op; the PREVIOUS
                    # chunk's reduce (input long ready) is emitted between the
                    # product and its consumer so the m0123 write settles
                    # without stalling the in-order vector queue
                    ppv = ppc.rearrange("p (cp ct) dg r -> p cp ct dg r", ct=2)
                    m0123 = mpool.tile([128, 2, 2, R], bf16, tag="m0123", bufs=3)
                    nc.vector.tensor_mul(m0123, ppv[:, :, 0], ppv[:, :, 1])
                    # chunk ch-1's reduce and quartic run here: their inputs
                    # were written >=1 chunk ago, so the in-order vector
                    # queue neither stalls on gpsimd nor hits the SBUF
                    # write-settle penalty of a just-written operand
                    if sq_prev is not None:
                        emit_reduce(ch - 1, sq_prev)
                        emit_quartic(ch - 1, m_prev)
                    sq_prev = sq
                    m_prev = m0123
                emit_reduce(3, sq_prev)
                emit_quartic(3, m_prev)
                rms = small.tile([128, 4 * C * 2], f32, tag="rms")
                nc.scalar.activation(
                    out=rms,
                    in_=ssq.rearrange("p ch c dg -> p (ch c dg)"),
                    func=Act.Sqrt,
                    bias=eps_t,
                    scale=1.0 / R,
                )
                rstd = small.tile([128, 4, C, 2], f32, tag="rstd")
                nc.vector.reciprocal(
                    out=rstd, in_=rms.rearrange("p (ch c dg) -> p ch c dg", c=C, dg=2)
                )
                # scl2[p, gh] = gain * prod_c rstd ; gh = 2*ch + dg
                scl2a = small.tile([128, GH], f32, tag="scl2a")
                nc.vector.tensor_reduce(
                    out=scl2a,
                    in_=rstd.rearrange("p ch c dg -> p ch dg c"),
                    axis=mybir.AxisListType.X,
                    op=mybir.AluOpType.mult,
                )
                scl2 = small.tile([128, GH], f32, tag="scl2")
                nc.vector.tensor_mul(scl2, scl2a, gpair[:, h0 : h0 + GH])
                mg = mpool.tile([128, GH, R], bf16, tag="mg")
                scl2b = scl2.unsqueeze(2).broadcast_to([128, GH, R])
                nc.vector.tensor_mul(mg, mgall, scl2b)
                st["mg"] = mg

            def phase2(gi, st):
                o_t = otpool.tile([128, GH * D], bf16, tag="o_t")
                for half in range(2):
                    # one psum bank per half-group of nodes
                    op_ps = ops.tile([128, 512], f32, tag="op")
                    # residual: out += 0.25 * sum_c x, via 0.25*I stationary.
                    # c==0 starts (overwrites) the bank; FO matmuls then
                    # accumulate on top.
                    for c in range(C):
                        nc.tensor.matmul(
                            op_ps,
                            lhsT=idq,
                            rhs=st["x"][:, c, 512 * half : 512 * half + 512],
                            start=(c == 0),
                            stop=False,
                        )
                    mtp = mtps.tile([64, 512], bf16, tag="mtp")
                    for k in range(4):
                        gh = 4 * half + k
                        nc.tensor.matmul(
                            mtp[:, 128 * k : 128 * k + 128],
                            lhsT=st["mg"][:, gh, :],
                            rhs=identity,
                            is_transpose=True,
                        )
                    mt = mtpool.tile([64, 512], bf16, tag="mt")
                    nc.scalar.copy(out=mt, in_=mtp)
                    for k in range(4):
                        gh = 4 * half + k
                        for g2 in range(2):
                            j = 2 * gh + g2
                            u8, j8 = j // 8, j % 8
                            col = 64 * (2 * k + g2)
                            nc.tensor.matmul(
                                op_ps[:, col : col + 64],
                                lhsT=st["fo"][:, u8, 128 * j8 : 128 * j8 + 128],
                                rhs=mt[:, col : col + 64],
                                start=False,
                                stop=(k == 3 and g2 == 1),
                            )
                    nc.scalar.copy(
                        out=o_t[:, 512 * half : 512 * half + 512], in_=op_ps
                    )
                nc.sync.dma_start(out=out[gi], in_=o_t)

            def emit_all_groups():
                # software pipeline: phase1+stats of group gi+1 is emitted
                # before phase2 of group gi so the PE stream never drains
                # while gi's stats chain completes; loads prefetch 2 groups
                prev = load(0)
                phase1_stats(0, prev)
                nxt = load(1)
                for gi in range(NG):
                    nxt2 = load(gi + 2) if gi + 2 < NG else None
                    if nxt is not None:
                        phase1_stats(gi + 1, nxt)
                    phase2(gi, prev)
                    prev, nxt = nxt, nxt2

            if repeat > 1:
                with tc.For_i(0, repeat, 1):
                    emit_all_groups()
            else:
                emit_all_groups()

    nc.compile()
    return nc


def _get_nc(repeat=1):
    key = ("nc", repeat)
    if key not in _CACHE:
        _CACHE[key] = _build_nc(repeat)
    return _CACHE[key]


def _pack_x(x):
    # [B, N, C, D] -> [N//16, C, D, 1024] bf16 ; n = g*16 + j, col = j*64 + b
    a = np.asarray(x).reshape(B, N // 16, 16, C, D)
    a = np.transpose(a, (1, 3, 4, 2, 0))  # [g, c, i, j, b]
    return np.ascontiguousarray(a.reshape(N // 16, C, D, 1024)).astype(BF16)


def _pack_factors(factors):
    # [4, N, R, D] -> [C, N//16, D, 1024] bf16
    f = np.asarray(factors).reshape(C, N // 16, 16, R, D)
    f = np.transpose(f, (0, 1, 4, 2, 3))  # [c, g, i, j, r]
    return np.ascontiguousarray(f.reshape(C, N // 16, D, 1024)).astype(BF16)


def _pack_factor_out(factor_out):
    # [N, R, D] -> [N//8, R, 8*D] bf16
    q = np.asarray(factor_out).reshape(N // 8, 8, R, D)
    q = np.transpose(q, (0, 2, 1, 3))  # [oct, r, node8, o]
    return np.ascontiguousarray(q.reshape(N // 8, R, 8 * D)).astype(BF16)


def _unpack_out(res_t):
    # [NG, 128(o), GH*D] bf16 with col = gh*128 + g2*64 + b -> [B, NS, D] f32
    a = np.asarray(res_t).astype(np.float32).reshape(NG, 128, GH, 2, 64)
    a = np.transpose(a, (4, 0, 2, 3, 1))  # [b, gi, gh, g2, o]
    return np.ascontiguousarray(a.reshape(64, NS, D))


def kernel(x, factors, factor_out, gain):
    from concourse.bass_utils import run_bass_kernel_spmd

    nc = _get_nc()
    x_packed = _pack_x(x)
    f_packed = _pack_factors(factors)
    fo_packed = _pack_factor_out(factor_out)
    gain = np.ascontiguousarray(np.asarray(gain, dtype=np.float32))
    in_maps = []
    for k in range(NCORES):
        lo, hi = k * NS, (k + 1) * NS
        in_maps.append(
            {
                "xp": np.ascontiguousarray(x_packed[k * NG : (k + 1) * NG]),
                "factors_t": np.ascontiguousarray(f_packed[:, k * NG : (k + 1) * NG]),
                "factor_out_t": np.ascontiguousarray(
                    fo_packed[k * OCT : (k + 1) * OCT]
                ),
                "gain": gain[lo:hi],
            }
        )
    res = run_bass_kernel_spmd(nc, in_maps, core_ids=list(range(NCORES)))
    return np.concatenate(
        [_unpack_out(res.results[k]["out_t"]) for k in range(NCORES)], axis=1
    )


# revision 45
# speedup vs baseline: 1.2556x; 1.1659x over previous
"""CPQuadRankLayer Trainium2 kernel, bf16 datapath, host-prepacked layouts.

Math (per node n, batch b):
  P[b,c,r]  = sum_i x[b,n,c,i] * factors[c,n,r,i]
  p         = P / sqrt(mean_r P^2 + eps)
  merged    = p0*p1*p2*p3 * gain[n]
  out[b,o]  = sum_r merged[b,r] * factor_out[n,r,o] + mean_c x[b,n,c,o]

Distribution: nodes sharded 1024 -> 8 cores x 128 nodes (node-
independent: no collectives). All tensors are repacked host-side to
bf16 with >=2KiB contiguous runs so DMA traffic is halved vs fp32 and
every matmul runs at the 1-cycle/row bf16 rate. The residual mean_c x
is folded into the output PSUM accumulation as four identity-matmul
accumulations (0.25*I stationary), which removes the entire vector-add
chain. The quartic pair-products run on the otherwise idle GpSimd
engine. Output is stored bf16 and upcast on the host.
"""

import numpy as np
import ml_dtypes

BF16 = ml_dtypes.bfloat16

B = 64
N = 1024
C = 4
D = 128
R = 64
NCORES = 8
NS = N // NCORES  # nodes per core (128)
G = 16  # nodes per group
NH = NS // 2  # node pairs per core
GH = G // 2  # node pairs per group (8)
NG = NS // G  # groups per core (8)
OCT = NS // 8  # octets per core (16)
EPS = 1e-6

_CACHE = {}


def _build_nc(repeat=1):
    import concourse.bacc as bacc
    import concourse.tile as tile
    import concourse.mybir as mybir
    from concourse.masks import make_identity

    f32 = mybir.dt.float32
    bf16 = mybir.dt.bfloat16
    f16 = mybir.dt.float16
    Act = mybir.ActivationFunctionType

    nc = bacc.Bacc()
    # x pre-packed: [group, c, i, (node16, b)] bf16 -> 2KiB runs
    xp = nc.declare_dram_parameter("xp", [NG, C, D, 1024], bf16, isOutput=False)
    # factors pre-packed: [c, group, i, (node16, r)] bf16 -> 2KiB runs
    f = nc.declare_dram_parameter("factors_t", [C, NG, D, 1024], bf16, isOutput=False)
    # factor_out pre-packed: [octet, r, (node8, o)] bf16 -> 2KiB runs
    fo = nc.declare_dram_parameter("factor_out_t", [OCT, R, 8 * D], bf16, isOutput=False)
    gain = nc.declare_dram_parameter("gain", [NS, 1], f32, isOutput=False)
    # packed output: [group, o, (gh, g2, b)] bf16; host unpacks + upcasts
    out = nc.declare_dram_parameter("out_t", [NG, 128, GH * D], bf16, isOutput=True)

    xp_r = xp.rearrange("g c i w -> i g c w")
    f_r = f.rearrange("c g i w -> i g c w")
    fo_r = fo.rearrange("u r w -> r u w")

    with tile.TileContext(nc) as tc:
        with (
            tc.tile_pool(name="consts", bufs=1) as consts,
            tc.tile_pool(name="xpool", bufs=3) as xpool,
            tc.tile_pool(name="fpool", bufs=3) as fpool,
            tc.tile_pool(name="fopool", bufs=3) as fopool,
            tc.tile_pool(name="ppcpool", bufs=2) as ppcpool,
            tc.tile_pool(name="sqpool", bufs=2) as sqpool,
            tc.tile_pool(name="mpool", bufs=2) as mpool,
            tc.tile_pool(name="mtpool", bufs=4) as mtpool,
            tc.tile_pool(name="otpool", bufs=2) as otpool,
            tc.tile_pool(name="small", bufs=10) as small,
            tc.tile_pool(name="pps", bufs=3, space="PSUM") as pps,
            tc.tile_pool(name="mtps", bufs=2, space="PSUM") as mtps,
            tc.tile_pool(name="ops", bufs=3, space="PSUM") as ops,
        ):
            identity = consts.tile([128, 128], bf16)
            make_identity(nc, identity)
            # 0.25*I stationary for the residual mean_c accumulation
            idq = consts.tile([128, 128], bf16)
            nc.scalar.mul(idq, identity, 0.25)
            eps_t = consts.tile([128, 1], f32)
            nc.vector.memset(eps_t, EPS)

            # gpair[p, h] = gain[2h + (p >= 64)] via two K=1 outer products
            ones1 = consts.tile([1, 128], f32)
            nc.vector.memset(ones1, 1.0)
            g1 = consts.tile([1, NS], f32)
            nc.sync.dma_start(out=g1, in_=gain.rearrange("n o -> o n"))
            g1v = g1.rearrange("o (h g2) -> o h g2", g2=2)
            gpp = pps.tile([128, NH], f32, tag="pp")
            nc.tensor.matmul(gpp[0:64, :], lhsT=ones1[:, 0:64], rhs=g1v[:, :, 0])
            nc.tensor.matmul(gpp[64:128, :], lhsT=ones1[:, 0:64], rhs=g1v[:, :, 1])
            gpair = consts.tile([128, NH], f32)
            nc.any.tensor_copy(gpair, gpp)

            def load(gi):
                st = {}
                xt = xpool.tile([128, C, 1024], bf16, tag="xt")
                nc.sync.dma_start(out=xt, in_=xp_r[:, gi])
                ft = fpool.tile([128, C, 1024], bf16, tag="ft")
                nc.scalar.dma_start(out=ft, in_=f_r[:, gi])
                fot = fopool.tile([R, 2, 8 * D], bf16, tag="fot")
                nc.scalar.dma_start(out=fot, in_=fo_r[:, 2 * gi : 2 * gi + 2])
                st["x"], st["f"], st["fo"] = xt, ft, fot
                return st

            def phase1_stats(gi, st):
                h0 = gi * GH
                # ssq[p, ch, c, dg]
                ssq = small.tile([128, 4, C, 2], f32, tag="ssq")
                mgall = mpool.tile([128, GH, R], bf16, tag="mgall")
                for ch in range(4):
                    # pp[p, c, dg, r] so downstream slices are contiguous
                    pp = pps.tile([128, C, 2, R], f32, tag="pp")
                    for dg in range(2):
                        gh = 2 * ch + dg
                        for c in range(C):
                            for g2 in range(2):
                                j = 2 * gh + g2
                                nc.tensor.matmul(
                                    pp[64 * g2 : 64 * g2 + 64, c, dg, :],
                                    lhsT=st["x"][:, c, 64 * j : 64 * j + 64],
                                    rhs=st["f"][:, c, 64 * j : 64 * j + 64],
                                )
                    # single psum->sbuf eviction (bf16); all stats read SBUF
                    # (gpsimd cannot touch PSUM, DVE only one PSUM operand)
                    ppc = ppcpool.tile([128, C, 2, R], bf16, tag="ppc")
                    nc.scalar.copy(out=ppc, in_=pp)
                    # squares on gpsimd (SBUF-only engine), reduce on vector
                    sq = sqpool.tile([128, C, 2, R], bf16, tag="sq")
                    nc.gpsimd.tensor_mul(sq, ppc, ppc)
                    nc.vector.reduce_sum(
                        out=ssq[:, ch],
                        in_=sq.rearrange("p c dg r -> p (c dg) r"),
                        axis=mybir.AxisListType.X,
                    )
                    # both pair products in one 2-byte-mode op, then the quartic
                    ppv = ppc.rearrange("p (cp ct) dg r -> p cp ct dg r", ct=2)
                    m0123 = mpool.tile([128, 2, 2, R], bf16, tag="m0123")
                    nc.vector.tensor_mul(m0123, ppv[:, :, 0], ppv[:, :, 1])
                    nc.vector.tensor_mul(
                        mgall[:, 2 * ch : 2 * ch + 2, :], m0123[:, 0], m0123[:, 1]
                    )
                rms = small.tile([128, 4 * C * 2], f32, tag="rms")
                nc.scalar.activation(
                    out=rms,
                    in_=ssq.rearrange("p ch c dg -> p (ch c dg)"),
                    func=Act.Sqrt,
                    bias=eps_t,
                    scale=1.0 / R,
                )
                rstd = small.tile([128, 4, C, 2], f32, tag="rstd")
                nc.vector.reciprocal(
                    out=rstd, in_=rms.rearrange("p (ch c dg) -> p ch c dg", c=C, dg=2)
                )
                # scl2[p, gh] = gain * prod_c rstd ; gh = 2*ch + dg
                scl2a = small.tile([128, GH], f32, tag="scl2a")
                nc.vector.tensor_reduce(
                    out=scl2a,
                    in_=rstd.rearrange("p ch c dg -> p ch dg c"),
                    axis=mybir.AxisListType.X,
                    op=mybir.AluOpType.mult,
                )
                scl2 = small.tile([128, GH], f32, tag="scl2")
                nc.vector.tensor_mul(scl2, scl2a, gpair[:, h0 : h0 + GH])
                mg = mpool.tile([128, GH, R], bf16, tag="mg")
                scl2b = scl2.unsqueeze(2).broadcast_to([128, GH, R])
                nc.vector.tensor_mul(mg, mgall, scl2b)
                st["mg"] = mg

            def phase2(gi, st):
                o_t = otpool.tile([128, GH * D], bf16, tag="o_t")
                for half in range(2):
                    # one psum bank per half-group of nodes
                    op_ps = ops.tile([128, 512], f32, tag="op")
                    # residual: out += 0.25 * sum_c x, via 0.25*I stationary.
                    # c==0 starts (overwrites) the bank; FO matmuls then
                    # accumulate on top.
                    for c in range(C):
                        nc.tensor.matmul(
                            op_ps,
                            lhsT=idq,
                            rhs=st["x"][:, c, 512 * half : 512 * half + 512],
                            start=(c == 0),
                            stop=False,
                        )
                    mtp = mtps.tile([64, 512], bf16, tag="mtp")
                    for k in range(4):
                        gh = 4 * half + k
                        nc.tensor.matmul(
                            mtp[:, 128 * k : 128 * k + 128],
                            lhsT=st["mg"][:, gh, :],
                            rhs=identity,
                            is_transpose=True,
                        )
                    mt = mtpool.tile([64, 512], bf16, tag="mt")
                    nc.scalar.copy(out=mt, in_=mtp)
                    for k in range(4):
                        gh = 4 * half + k
                        for g2 in range(2):
                            j = 2 * gh + g2
                            u8, j8 = j // 8, j % 8
                            col = 64 * (2 * k + g2)
                            nc.tensor.matmul(
                                op_ps[:, col : col + 64],
                                lhsT=st["fo"][:, u8, 128 * j8 : 128 * j8 + 128],
                                rhs=mt[:, col : col + 64],
                                start=False,
                                stop=(k == 3 and g2 == 1),
                            )
                    nc.scalar.copy(
                        out=o_t[:, 512 * half : 512 * half + 512], in_=op_ps
                    )
                nc.sync.dma_start(out=out[gi], in_=o_t)

            def emit_all_groups():
                # software pipeline: phase1+stats of group gi+1 is emitted
                # before phase2 of group gi so the PE stream never drains
                # while gi's stats chain completes; loads prefetch 2 groups
                prev = load(0)
                phase1_stats(0, prev)
                nxt = load(1)
                for gi in range(NG):
                    nxt2 = load(gi + 2) if gi + 2 < NG else None
                    if nxt is not None:
                        phase1_stats(gi + 1, nxt)
                    phase2(gi, prev)
                    prev, nxt = nxt, nxt2

            if repeat > 1:
                with tc.For_i(0, repeat, 1):
                    emit_all_groups()
            else:
                emit_all_groups()

    nc.compile()
    return nc


def _get_nc(repeat=1):
    key = ("nc", repeat)
    if key not in _CACHE:
        _CACHE[key] = _build_nc(repeat)
    return _CACHE[key]


def _pack_x(x):
    # [B, N, C, D] -> [N//16, C, D, 1024] bf16 ; n = g*16 + j, col = j*64 + b
    a = np.asarray(x).reshape(B, N // 16, 16, C, D)
    a = np.transpose(a, (1, 3, 4, 2, 0))  # [g, c, i, j, b]
    return np.ascontiguousarray(a.reshape(N // 16, C, D, 1024)).astype(BF16)


def _pack_factors(factors):
    # [4, N, R, D] -> [C, N//16, D, 1024] bf16
    f = np.asarray(factors).reshape(C, N // 16, 16, R, D)
    f = np.transpose(f, (0, 1, 4, 2, 3))  # [c, g, i, j, r]
    return np.ascontiguousarray(f.reshape(C, N // 16, D, 1024)).astype(BF16)


def _pack_factor_out(factor_out):
    # [N, R, D] -> [N//8, R, 8*D] bf16
    q = np.asarray(factor_out).reshape(N // 8, 8, R, D)
    q = np.transpose(q, (0, 2, 1, 3))  # [oct, r, node8, o]
    return np.ascontiguousarray(q.reshape(N // 8, R, 8 * D)).astype(BF16)


def _unpack_out(res_t):
    # [NG, 128(o), GH*D] bf16 with col = gh*128 + g2*64 + b -> [B, NS, D] f32
    a = np.asarray(res_t).astype(np.float32).reshape(NG, 128, GH, 2, 64)
    a = np.transpose(a, (4, 0, 2, 3, 1))  # [b, gi, gh, g2, o]
    return np.ascontiguousarray(a.reshape(64, NS, D))


def kernel(x, factors, factor_out, gain):
    from concourse.bass_utils import run_bass_kernel_spmd

    nc = _get_nc()
    x_packed = _pack_x(x)
    f_packed = _pack_factors(factors)
    fo_packed = _pack_factor_out(factor_out)
    gain = np.ascontiguousarray(np.asarray(gain, dtype=np.float32))
    in_maps = []
    for k in range(NCORES):
        lo, hi = k * NS, (k + 1) * NS
        in_maps.append(
            {
                "xp": np.ascontiguousarray(x_packed[k * NG : (k + 1) * NG]),
                "factors_t": np.ascontiguousarray(f_packed[:, k * NG : (k + 1) * NG]),
                "factor_out_t": np.ascontiguousarray(
                    fo_packed[k * OCT : (k + 1) * OCT]
                ),
                "gain": gain[lo:hi],
            }
        )
    res = run_bass_kernel_spmd(nc, in_maps, core_ids=list(range(NCORES)))
    return np.concatenate(
        [_unpack_out(res.results[k]["out_t"]) for k in range(NCORES)], axis=1
    )
